# revision 6
# baseline (speedup 1.0000x reference)
"""Trainium2 Bass kernel: CRF Viterbi decode (torchcrf CRF.decode semantics).

Problem: B=512, T=512, K=64. Data-parallel over batch across 8 NeuronCores
(64 batch rows per core). Bit-exact with the reference.

Design (~3.8x faster than the naive 6-big-op/step forward):
  Forward (per step ~5.5us, all on the vector engine, which stays ~97%%
  busy): scores kept in split-j layout [128p, 32] (partition p = b + 64*jh
  holds j-half jh of batch row b), so the two big DVE ops shrink to 2048
  elems/partition:
      z[p, jl, i]   = srep[p, i] + trans[i, jh*32+jl]   (tensor_add)
      raw[p, jl]    = max_i z                           (tensor_reduce X)
  srep ("s replicated": every partition holds the full 64-wide s_t) is
  rebuilt each step by 4 small quadrant tensor_adds that simultaneously
  apply the emission add (srep = raw + e_t); two of them read across
  partition groups (cross-partition-offset APs). The [b,i]-layout score
  snapshot for the backtrace is one SBUF->SBUF DMA per step, off the DVE
  chain.

  No argmax/history is computed in the forward pass. The backtrace
  recomputes the argmax only along the decoded path (~2.8us/step,
  [64,64]-sized ops):
      onehT = oneh^T                 (PE transpose via identity)
      tcol  = onehT.T @ trans^T      (PE: gathers trans[:, tag_{t+1}])
      cand  = (shist_t + tcol) + e_sel    (exact reference add order)
      tag_t = first-occurrence argmax_i cand   (MAX8 + FIND_INDEX8;
              ties return ascending indices, so index [0] is the
              reference's first-occurrence argmax)
  e_sel = e_{t+1}[b, tag_{t+1}] via a one-hot multiply-accumulate.

Exactness: reference cand[b,i,j] = (s[b,i] + trans[i,j]) + e[t,b,j], max
over i, first-occurrence argmax. Deferring the emission add past the max
is bit-exact for the max (round is monotone; e is constant over i), and
the backtrace recomputes cand in the reference's exact add order, so both
scores and tie resolution match the reference bit-for-bit. The PE gather
is exact because its lhsT is a one-hot matrix (single x*1.0 term).
"""

import numpy as np

import concourse.bacc as bacc
import concourse.mybir as mybir
import concourse.tile as tile
from concourse.bass_utils import run_bass_kernel_spmd

B, T, K = 512, 512, 64
NCORES = 8
BC = B // NCORES  # 64 batch rows per core
KH = K // 2       # 32, the j-half width

F32 = mybir.dt.float32
I32 = mybir.dt.int32
AX = mybir.AxisListType.X
OP = mybir.AluOpType

FCH = 32   # forward emission chunk, steps
BCH = 32   # backtrace emission chunk, steps


def build_nc(t_run=T):
    nc = bacc.Bacc("TRN2", target_bir_lowering=False, debug=False)

    em_split = nc.dram_tensor("em_split", [128, t_run * KH], F32,
                              kind="ExternalInput")
    em_full = nc.dram_tensor("em_full", [BC, t_run * K], F32,
                             kind="ExternalInput")
    ttsplit = nc.dram_tensor("ttsplit", [128, KH * K], F32,
                             kind="ExternalInput")
    start_split = nc.dram_tensor("start_split", [128, KH], F32,
                                 kind="ExternalInput")
    transt = nc.dram_tensor("transt", [K, K], F32, kind="ExternalInput")
    endr = nc.dram_tensor("endr", [1, K], F32, kind="ExternalInput")
    wcoefr = nc.dram_tensor("wcoefr", [1, K], F32, kind="ExternalInput")
    iotar = nc.dram_tensor("iotar", [1, K], F32, kind="ExternalInput")
    ident = nc.dram_tensor("ident", [K, K], F32, kind="ExternalInput")
    tags = nc.dram_tensor("tags", [BC, t_run], I32, kind="ExternalOutput")

    nfch = t_run // FCH
    nbch = t_run // BCH

    with tile.TileContext(nc) as tc:
        with (
            tc.tile_pool(name="persist", bufs=1) as pp,
            tc.tile_pool(name="zwork", bufs=1) as zp,
            tc.tile_pool(name="fem", bufs=2) as fep,
            tc.tile_pool(name="bem", bufs=2) as bep,
        ):
            tts = pp.tile_from(ttsplit[:, :])
            startS = pp.tile_from(start_split[:, :])
            transTS = pp.tile_from(transt[:, :])
            endS = pp.tile_from(endr[0:1, :].broadcast_to([BC, K]))
            wcoefS = pp.tile_from(wcoefr[0:1, :].broadcast_to([BC, K]))
            iotaS = pp.tile_from(iotar[0:1, :].broadcast_to([BC, K]))
            identS = pp.tile_from(ident[:, :])
            tts3 = tts[:, :].rearrange("p (j i) -> p j i", i=K)

            rawa = pp.tile([128, KH], F32)
            rawb = pp.tile([128, KH], F32)
            rawp = [rawa, rawb]
            srepa = pp.tile([128, K], F32)
            srepb = pp.tile([128, K], F32)
            srepp = [srepa, srepb]
            shist = pp.tile([BC, t_run * K], F32)   # s_0 .. s_{T-1}
            tagsf = pp.tile([BC, t_run], F32)
            tagsi = pp.tile([BC, t_run], I32)
            oneh = pp.tile([BC, K], F32)
            onehT = pp.tile([BC, K], F32)
            c1 = pp.tile([BC, K], F32)
            w1 = pp.tile([BC, K], F32)
            fin = pp.tile([BC, K], F32)
            m1 = pp.tile([BC, 1], F32)
            pw = pp.tile([BC, 1], F32)
            esel = pp.tile([BC, 1], F32)
            junk = pp.tile([BC, K], F32)
            mx8 = pp.tile([BC, 8], F32)
            junk2 = pp.tile([BC, K], F32)
            mi8 = pp.tile([BC, 8], mybir.dt.uint32)
            idxf = pp.tile([BC, 1], F32)

            # ---------------- forward ----------------
            # All-DVE forward. srep ("s replicated": every partition holds
            # the full 64-wide score vector s_t) is rebuilt each step by 4
            # quadrant tensor_adds that simultaneously apply the emission
            # add; two of them read across partition groups (cross-offset
            # APs, verified on HW). The [b,i]-layout history snapshot is a
            # single SBUF->SBUF DMA of srep's lower half, off the DVE chain.
            fetiles = {}

            def load_f(c):
                if 0 <= c < nfch and c not in fetiles:
                    ft = fep.tile([128, FCH * KH], F32, tag="fe")
                    nc.sync.dma_start(
                        ft[:, :],
                        em_split[:, c * FCH * KH:(c + 1) * FCH * KH])
                    fetiles[c] = ft

            load_f(0)
            for t in range(t_run):
                c, r = divmod(t, FCH)
                if r == 0:
                    load_f(c + 1)
                fechunk = fetiles[c]
                e_t = fechunk[:, r * KH:(r + 1) * KH]
                raw = startS if t == 0 else rawp[t % 2]
                srep = srepp[t % 2]
                # srep[p, :] = s_t (full width) = raw_t + e_t, quadrant-wise
                nc.vector.tensor_add(srep[0:BC, 0:KH], raw[0:BC, :],
                                     e_t[0:BC, :])
                nc.vector.tensor_add(srep[BC:128, KH:K], raw[BC:128, :],
                                     e_t[BC:128, :])
                nc.vector.tensor_add(srep[0:BC, KH:K], raw[BC:128, :],
                                     e_t[BC:128, :])
                nc.vector.tensor_add(srep[BC:128, 0:KH], raw[0:BC, :],
                                     e_t[0:BC, :])
                nc.sync.dma_start(shist[:, t * K:(t + 1) * K],
                                  srep[0:BC, :])
                if t < t_run - 1:
                    z = zp.tile([128, KH * K], F32, tag="z")
                    z3 = z[:, :].rearrange("p (j i) -> p j i", i=K)
                    srep_b3 = srep[:, :].unsqueeze(1).broadcast_to(
                        [128, KH, K])
                    nc.vector.tensor_add(z3, srep_b3, tts3)
                    nc.vector.tensor_reduce(rawp[(t + 1) % 2][:, :], z3,
                                            axis=AX, op=OP.max)

            nc.vector.tensor_add(fin[:, :], srepp[(t_run - 1) % 2][0:BC, :],
                                 endS[:, :])

            # ---------------- final argmax ----------------
            nc.vector.tensor_reduce(m1[:, :], fin[:, :], axis=AX, op=OP.max)
            nc.vector.scalar_tensor_tensor(
                out=w1[:, :], in0=fin[:, :], scalar=m1[:, 0:1],
                in1=wcoefS[:, :], op0=OP.is_ge, op1=OP.mult)
            nc.vector.tensor_reduce(pw[:, :], w1[:, :], axis=AX, op=OP.max)
            nc.vector.tensor_single_scalar(oneh[:, :], w1[:, :], pw[:, 0:1],
                                           op=OP.is_equal)
            nc.vector.tensor_scalar(tagsf[:, t_run - 1:t_run], pw[:, :],
                                    -1.0, 64.0, op0=OP.mult, op1=OP.add)

            # ---------------- backtrace ----------------
            betiles = {}

            def load_b(c):
                if 0 <= c < nbch and c not in betiles:
                    bt = bep.tile([BC, BCH * K], F32, tag="be")
                    nc.sync.dma_start(
                        bt[:, :],
                        em_full[:, c * BCH * K:(c + 1) * BCH * K])
                    betiles[c] = bt

            nc.vector.memset(mx8[:, :], 0.0)
            with tc.tile_pool(name="psumB", bufs=2, space="PSUM") as psb:
                load_b(nbch - 1)
                for c in range(nbch - 1, -1, -1):
                    load_b(c - 1)
                    bchunk = betiles[c]
                    t1_lo = max(c * BCH, 1)
                    for t1 in range((c + 1) * BCH - 1, t1_lo - 1, -1):
                        t = t1 - 1  # computing tag_t from tag_{t+1}=tag_{t1}
                        rr = t1 - c * BCH
                        e_t1 = bchunk[:, rr * K:(rr + 1) * K]
                        pG = psb.tile([BC, K], F32, tag="pg")
                        pT = psb.tile([BC, K], F32, tag="pt")
                        nc.tensor.transpose(pT[:, :], oneh[:, :], identS)
                        nc.vector.tensor_copy(onehT[:, :], pT[:, :])
                        nc.tensor.matmul(pG[:, :], onehT[:, :],
                                         transTS[:, :], start=True, stop=True)
                        nc.vector.scalar_tensor_tensor(
                            out=junk[:, :], in0=oneh[:, :], scalar=1.0,
                            in1=e_t1, op0=OP.mult, op1=OP.mult,
                            accum_out=esel[:, :])
                        # known max of cand: m_sel = s_{t+1}[b, tag_{t+1}]
                        # (= round(max(c1)+esel) by monotone rounding); goes
                        # into find_index8's in_max[0], removing MAX8 from
                        # the serial chain.
                        nc.vector.scalar_tensor_tensor(
                            out=junk2[:, :], in0=oneh[:, :], scalar=1.0,
                            in1=shist[:, (t + 1) * K:(t + 2) * K],
                            op0=OP.mult, op1=OP.mult,
                            accum_out=mx8[:, 0:1])
                        nc.vector.tensor_add(
                            c1[:, :], shist[:, t * K:(t + 1) * K], pG[:, :])
                        nc.vector.tensor_single_scalar(
                            c1[:, :], c1[:, :], esel[:, 0:1], op=OP.add)
                        nc.vector.max_index(mi8[:, :], mx8[:, :], c1[:, :])
                        nc.vector.tensor_copy(idxf[:, :], mi8[:, 0:1])
                        nc.vector.tensor_single_scalar(
                            oneh[:, :], iotaS[:, :], idxf[:, 0:1],
                            op=OP.is_equal)
                        nc.vector.tensor_copy(tagsf[:, t:t + 1], idxf[:, :])

            nc.vector.tensor_copy(tagsi[:, :], tagsf[:, :])
            nc.sync.dma_start(tags[:, :], tagsi[:, :])

    nc.compile()
    return nc


def make_in_maps(emissions, start_transitions, end_transitions, transitions,
                 t_run=T):
    em = np.asarray(emissions, dtype=np.float32)
    start = np.asarray(start_transitions, dtype=np.float32)
    end = np.asarray(end_transitions, dtype=np.float32)
    trans = np.asarray(transitions, dtype=np.float32)

    transT = np.ascontiguousarray(trans.T)
    ttsplit = np.concatenate([
        np.tile(transT[0:KH, :].reshape(1, -1), (64, 1)),
        np.tile(transT[KH:K, :].reshape(1, -1), (64, 1)),
    ], axis=0).astype(np.float32)
    start_split = np.concatenate([
        np.tile(start[None, 0:KH], (64, 1)),
        np.tile(start[None, KH:K], (64, 1)),
    ], axis=0).astype(np.float32)

    base = {
        "ttsplit": np.ascontiguousarray(ttsplit),
        "start_split": np.ascontiguousarray(start_split),
        "transt": transT,
        "endr": np.ascontiguousarray(end[None, :]),
        "wcoefr": (K - np.arange(K, dtype=np.float32))[None, :],
        "iotar": np.arange(K, dtype=np.float32)[None, :],
        "ident": np.eye(K, dtype=np.float32),
    }
    in_maps = []
    for cix in range(NCORES):
        emc = em[cix * BC:(cix + 1) * BC, :t_run]  # [BC, t_run, K]
        m = dict(base)
        m["em_full"] = np.ascontiguousarray(emc.reshape(BC, t_run * K))
        m["em_split"] = np.ascontiguousarray(
            np.concatenate([emc[:, :, 0:KH], emc[:, :, KH:K]],
                           axis=0).reshape(128, t_run * KH))
        in_maps.append(m)
    return in_maps


def kernel(emissions, attn_mask, start_transitions, end_transitions,
           transitions):
    # attn_mask is all-ones for this problem (spec fill=ones); with an
    # all-True mask the reference's mask logic is a no-op.
    nc = build_nc(T)
    in_maps = make_in_maps(emissions, start_transitions, end_transitions,
                           transitions, T)
    res = run_bass_kernel_spmd(nc, in_maps, list(range(NCORES))).results
    out = np.concatenate([res[c]["tags"] for c in range(NCORES)], axis=0)
    return out.astype(np.int32)


if __name__ == "__main__":
    rng = np.random.default_rng(0)
    em = rng.standard_normal((B, T, K)).astype(np.float32)
    am = np.ones((B, T), np.int32)
    st = (rng.standard_normal(K) * 0.1).astype(np.float32)
    en = (rng.standard_normal(K) * 0.1).astype(np.float32)
    tr = (rng.standard_normal((K, K)) * 0.1).astype(np.float32)
    print(kernel(em, am, st, en, tr)[:2, :8])


# revision 7
# speedup vs baseline: 1.0275x; 1.0275x over previous
"""Trainium2 Bass kernel: CRF Viterbi decode (torchcrf CRF.decode semantics).

Problem: B=512, T=512, K=64. Data-parallel over batch across 8 NeuronCores
(64 batch rows per core). Bit-exact with the reference.

Design (~3.8x faster than the naive 6-big-op/step forward):
  Forward (per step ~5.5us, all on the vector engine, which stays ~97%%
  busy): scores kept in split-j layout [128p, 32] (partition p = b + 64*jh
  holds j-half jh of batch row b), so the two big DVE ops shrink to 2048
  elems/partition:
      z[p, jl, i]   = srep[p, i] + trans[i, jh*32+jl]   (tensor_add)
      raw[p, jl]    = max_i z                           (tensor_reduce X)
  srep ("s replicated": every partition holds the full 64-wide s_t) is
  rebuilt each step by 4 small quadrant tensor_adds that simultaneously
  apply the emission add (srep = raw + e_t); two of them read across
  partition groups (cross-partition-offset APs). The [b,i]-layout score
  snapshot for the backtrace is one SBUF->SBUF DMA per step, off the DVE
  chain.

  No argmax/history is computed in the forward pass. The backtrace
  recomputes the argmax only along the decoded path (~2.8us/step,
  [64,64]-sized ops):
      onehT = oneh^T                 (PE transpose via identity)
      tcol  = onehT.T @ trans^T      (PE: gathers trans[:, tag_{t+1}])
      cand  = (shist_t + tcol) + e_sel    (exact reference add order)
      tag_t = first-occurrence argmax_i cand, via FIND_INDEX8 fed with
              the KNOWN max: max(cand) = s_{t+1}[b, tag_{t+1}] exactly
              (monotone rounding), extracted off the critical path by a
              one-hot accumulate — no MAX8 on the serial chain.
  e_sel = e_{t+1}[b, tag_{t+1}] via a one-hot multiply-accumulate.

Exactness: reference cand[b,i,j] = (s[b,i] + trans[i,j]) + e[t,b,j], max
over i, first-occurrence argmax. Deferring the emission add past the max
is bit-exact for the max (round is monotone; e is constant over i), and
the backtrace recomputes cand in the reference's exact add order, so both
scores and tie resolution match the reference bit-for-bit. The PE gather
is exact because its lhsT is a one-hot matrix (single x*1.0 term).
"""

import numpy as np

import concourse.bacc as bacc
import concourse.mybir as mybir
import concourse.tile as tile
from concourse.bass_utils import run_bass_kernel_spmd

B, T, K = 512, 512, 64
NCORES = 8
BC = B // NCORES  # 64 batch rows per core
KH = K // 2       # 32, the j-half width

F32 = mybir.dt.float32
I32 = mybir.dt.int32
AX = mybir.AxisListType.X
OP = mybir.AluOpType

FCH = 32   # forward emission chunk, steps
BCH = 32   # backtrace emission chunk, steps


def build_nc(t_run=T):
    nc = bacc.Bacc("TRN2", target_bir_lowering=False, debug=False)

    em_split = nc.dram_tensor("em_split", [128, t_run * KH], F32,
                              kind="ExternalInput")
    em_full = nc.dram_tensor("em_full", [BC, t_run * K], F32,
                             kind="ExternalInput")
    ttsplit = nc.dram_tensor("ttsplit", [128, KH * K], F32,
                             kind="ExternalInput")
    start_split = nc.dram_tensor("start_split", [128, KH], F32,
                                 kind="ExternalInput")
    transt = nc.dram_tensor("transt", [K, K], F32, kind="ExternalInput")
    endr = nc.dram_tensor("endr", [1, K], F32, kind="ExternalInput")
    wcoefr = nc.dram_tensor("wcoefr", [1, K], F32, kind="ExternalInput")
    iotar = nc.dram_tensor("iotar", [1, K], F32, kind="ExternalInput")
    ident = nc.dram_tensor("ident", [K, K], F32, kind="ExternalInput")
    tags = nc.dram_tensor("tags", [BC, t_run], I32, kind="ExternalOutput")

    nfch = t_run // FCH
    nbch = t_run // BCH

    with tile.TileContext(nc) as tc:
        with (
            tc.tile_pool(name="persist", bufs=1) as pp,
            tc.tile_pool(name="zwork", bufs=1) as zp,
            tc.tile_pool(name="fem", bufs=2) as fep,
            tc.tile_pool(name="bem", bufs=2) as bep,
        ):
            tts = pp.tile_from(ttsplit[:, :])
            startS = pp.tile_from(start_split[:, :])
            transTS = pp.tile_from(transt[:, :])
            endS = pp.tile_from(endr[0:1, :].broadcast_to([BC, K]))
            wcoefS = pp.tile_from(wcoefr[0:1, :].broadcast_to([BC, K]))
            iotaS = pp.tile_from(iotar[0:1, :].broadcast_to([BC, K]))
            identS = pp.tile_from(ident[:, :])
            tts3 = tts[:, :].rearrange("p (j i) -> p j i", i=K)

            rawa = pp.tile([128, KH], F32)
            rawb = pp.tile([128, KH], F32)
            rawp = [rawa, rawb]
            srepa = pp.tile([128, K], F32)
            srepb = pp.tile([128, K], F32)
            srepp = [srepa, srepb]
            shist = pp.tile([BC, t_run * K], F32)   # s_0 .. s_{T-1}
            tagsf = pp.tile([BC, t_run], F32)
            tagsi = pp.tile([BC, t_run], I32)
            oneh = pp.tile([BC, K], F32)
            onehT = pp.tile([BC, K], F32)
            c1 = pp.tile([BC, K], F32)
            w1 = pp.tile([BC, K], F32)
            fin = pp.tile([BC, K], F32)
            m1 = pp.tile([BC, 1], F32)
            pw = pp.tile([BC, 1], F32)
            esel = pp.tile([BC, 1], F32)
            junk = pp.tile([BC, K], F32)
            mx8 = pp.tile([BC, 8], F32)
            junk2 = pp.tile([BC, K], F32)
            mi8 = pp.tile([BC, 8], mybir.dt.uint32)
            idxf = pp.tile([BC, 1], F32)

            # ---------------- forward ----------------
            # All-DVE forward. srep ("s replicated": every partition holds
            # the full 64-wide score vector s_t) is rebuilt each step by 4
            # quadrant tensor_adds that simultaneously apply the emission
            # add; two of them read across partition groups (cross-offset
            # APs, verified on HW). The [b,i]-layout history snapshot is a
            # single SBUF->SBUF DMA of srep's lower half, off the DVE chain.
            fetiles = {}

            def load_f(c):
                if 0 <= c < nfch and c not in fetiles:
                    ft = fep.tile([128, FCH * KH], F32, tag="fe")
                    nc.sync.dma_start(
                        ft[:, :],
                        em_split[:, c * FCH * KH:(c + 1) * FCH * KH])
                    fetiles[c] = ft

            load_f(0)
            for t in range(t_run):
                c, r = divmod(t, FCH)
                if r == 0:
                    load_f(c + 1)
                fechunk = fetiles[c]
                e_t = fechunk[:, r * KH:(r + 1) * KH]
                raw = startS if t == 0 else rawp[t % 2]
                srep = srepp[t % 2]
                # srep[p, :] = s_t (full width) = raw_t + e_t, quadrant-wise
                nc.vector.tensor_add(srep[0:BC, 0:KH], raw[0:BC, :],
                                     e_t[0:BC, :])
                nc.vector.tensor_add(srep[BC:128, KH:K], raw[BC:128, :],
                                     e_t[BC:128, :])
                nc.vector.tensor_add(srep[0:BC, KH:K], raw[BC:128, :],
                                     e_t[BC:128, :])
                nc.vector.tensor_add(srep[BC:128, 0:KH], raw[0:BC, :],
                                     e_t[0:BC, :])
                nc.sync.dma_start(shist[:, t * K:(t + 1) * K],
                                  srep[0:BC, :])
                if t < t_run - 1:
                    z = zp.tile([128, KH * K], F32, tag="z")
                    z3 = z[:, :].rearrange("p (j i) -> p j i", i=K)
                    srep_b3 = srep[:, :].unsqueeze(1).broadcast_to(
                        [128, KH, K])
                    nc.vector.tensor_add(z3, srep_b3, tts3)
                    nc.vector.tensor_reduce(rawp[(t + 1) % 2][:, :], z3,
                                            axis=AX, op=OP.max)

            nc.vector.tensor_add(fin[:, :], srepp[(t_run - 1) % 2][0:BC, :],
                                 endS[:, :])

            # ---------------- final argmax ----------------
            nc.vector.tensor_reduce(m1[:, :], fin[:, :], axis=AX, op=OP.max)
            nc.vector.scalar_tensor_tensor(
                out=w1[:, :], in0=fin[:, :], scalar=m1[:, 0:1],
                in1=wcoefS[:, :], op0=OP.is_ge, op1=OP.mult)
            nc.vector.tensor_reduce(pw[:, :], w1[:, :], axis=AX, op=OP.max)
            nc.vector.tensor_single_scalar(oneh[:, :], w1[:, :], pw[:, 0:1],
                                           op=OP.is_equal)
            nc.vector.tensor_scalar(tagsf[:, t_run - 1:t_run], pw[:, :],
                                    -1.0, 64.0, op0=OP.mult, op1=OP.add)

            # ---------------- backtrace ----------------
            betiles = {}

            def load_b(c):
                if 0 <= c < nbch and c not in betiles:
                    bt = bep.tile([BC, BCH * K], F32, tag="be")
                    nc.sync.dma_start(
                        bt[:, :],
                        em_full[:, c * BCH * K:(c + 1) * BCH * K])
                    betiles[c] = bt

            nc.vector.memset(mx8[:, :], 0.0)
            with tc.tile_pool(name="psumB", bufs=2, space="PSUM") as psb:
                load_b(nbch - 1)
                for c in range(nbch - 1, -1, -1):
                    load_b(c - 1)
                    bchunk = betiles[c]
                    t1_lo = max(c * BCH, 1)
                    for t1 in range((c + 1) * BCH - 1, t1_lo - 1, -1):
                        t = t1 - 1  # computing tag_t from tag_{t+1}=tag_{t1}
                        rr = t1 - c * BCH
                        e_t1 = bchunk[:, rr * K:(rr + 1) * K]
                        pG = psb.tile([BC, K], F32, tag="pg")
                        pT = psb.tile([BC, K], F32, tag="pt")
                        nc.tensor.transpose(pT[:, :], oneh[:, :], identS)
                        nc.vector.tensor_copy(onehT[:, :], pT[:, :])
                        nc.tensor.matmul(pG[:, :], onehT[:, :],
                                         transTS[:, :], start=True, stop=True)
                        nc.vector.scalar_tensor_tensor(
                            out=junk[:, :], in0=oneh[:, :], scalar=1.0,
                            in1=e_t1, op0=OP.mult, op1=OP.mult,
                            accum_out=esel[:, :])
                        # known max of cand: m_sel = s_{t+1}[b, tag_{t+1}]
                        # (= round(max(c1)+esel) by monotone rounding); goes
                        # into find_index8's in_max[0], removing MAX8 from
                        # the serial chain.
                        nc.vector.scalar_tensor_tensor(
                            out=junk2[:, :], in0=oneh[:, :], scalar=1.0,
                            in1=shist[:, (t + 1) * K:(t + 2) * K],
                            op0=OP.mult, op1=OP.mult,
                            accum_out=mx8[:, 0:1])
                        nc.vector.tensor_add(
                            c1[:, :], shist[:, t * K:(t + 1) * K], pG[:, :])
                        nc.vector.tensor_single_scalar(
                            c1[:, :], c1[:, :], esel[:, 0:1], op=OP.add)
                        nc.vector.max_index(mi8[:, :], mx8[:, :], c1[:, :])
                        nc.vector.tensor_copy(idxf[:, :], mi8[:, 0:1])
                        nc.vector.tensor_single_scalar(
                            oneh[:, :], iotaS[:, :], idxf[:, 0:1],
                            op=OP.is_equal)
                        nc.vector.tensor_copy(tagsf[:, t:t + 1], idxf[:, :])

            nc.vector.tensor_copy(tagsi[:, :], tagsf[:, :])
            nc.sync.dma_start(tags[:, :], tagsi[:, :])

    nc.compile()
    return nc


def make_in_maps(emissions, start_transitions, end_transitions, transitions,
                 t_run=T):
    em = np.asarray(emissions, dtype=np.float32)
    start = np.asarray(start_transitions, dtype=np.float32)
    end = np.asarray(end_transitions, dtype=np.float32)
    trans = np.asarray(transitions, dtype=np.float32)

    transT = np.ascontiguousarray(trans.T)
    ttsplit = np.concatenate([
        np.tile(transT[0:KH, :].reshape(1, -1), (64, 1)),
        np.tile(transT[KH:K, :].reshape(1, -1), (64, 1)),
    ], axis=0).astype(np.float32)
    start_split = np.concatenate([
        np.tile(start[None, 0:KH], (64, 1)),
        np.tile(start[None, KH:K], (64, 1)),
    ], axis=0).astype(np.float32)

    base = {
        "ttsplit": np.ascontiguousarray(ttsplit),
        "start_split": np.ascontiguousarray(start_split),
        "transt": transT,
        "endr": np.ascontiguousarray(end[None, :]),
        "wcoefr": (K - np.arange(K, dtype=np.float32))[None, :],
        "iotar": np.arange(K, dtype=np.float32)[None, :],
        "ident": np.eye(K, dtype=np.float32),
    }
    in_maps = []
    for cix in range(NCORES):
        emc = em[cix * BC:(cix + 1) * BC, :t_run]  # [BC, t_run, K]
        m = dict(base)
        m["em_full"] = np.ascontiguousarray(emc.reshape(BC, t_run * K))
        m["em_split"] = np.ascontiguousarray(
            np.concatenate([emc[:, :, 0:KH], emc[:, :, KH:K]],
                           axis=0).reshape(128, t_run * KH))
        in_maps.append(m)
    return in_maps


def kernel(emissions, attn_mask, start_transitions, end_transitions,
           transitions):
    # attn_mask is all-ones for this problem (spec fill=ones); with an
    # all-True mask the reference's mask logic is a no-op.
    nc = build_nc(T)
    in_maps = make_in_maps(emissions, start_transitions, end_transitions,
                           transitions, T)
    res = run_bass_kernel_spmd(nc, in_maps, list(range(NCORES))).results
    out = np.concatenate([res[c]["tags"] for c in range(NCORES)], axis=0)
    return out.astype(np.int32)


if __name__ == "__main__":
    rng = np.random.default_rng(0)
    em = rng.standard_normal((B, T, K)).astype(np.float32)
    am = np.ones((B, T), np.int32)
    st = (rng.standard_normal(K) * 0.1).astype(np.float32)
    en = (rng.standard_normal(K) * 0.1).astype(np.float32)
    tr = (rng.standard_normal((K, K)) * 0.1).astype(np.float32)
    print(kernel(em, am, st, en, tr)[:2, :8])


# revision 8
# speedup vs baseline: 1.0286x; 1.0010x over previous
"""Trainium2 Bass kernel: CRF Viterbi decode (torchcrf CRF.decode semantics).

Problem: B=512, T=512, K=64. Data-parallel over batch across 8 NeuronCores
(64 batch rows per core). Bit-exact with the reference.

Design (~3.8x faster than the naive 6-big-op/step forward):
  Forward (per step ~5.5us, all on the vector engine, which stays ~97%%
  busy): scores kept in split-j layout [128p, 32] (partition p = b + 64*jh
  holds j-half jh of batch row b), so the two big DVE ops shrink to 2048
  elems/partition:
      z[p, jl, i]   = srep[p, i] + trans[i, jh*32+jl]   (tensor_add)
      raw[p, jl]    = max_i z                           (tensor_reduce X)
  srep ("s replicated": every partition holds the full 64-wide s_t) is
  rebuilt each step by 4 small quadrant tensor_adds that simultaneously
  apply the emission add (srep = raw + e_t); two of them read across
  partition groups (cross-partition-offset APs). The [b,i]-layout score
  snapshot for the backtrace is one SBUF->SBUF DMA per step, off the DVE
  chain.

  No argmax/history is computed in the forward pass. The backtrace
  recomputes the argmax only along the decoded path (~2.8us/step,
  [64,64]-sized ops):
      onehT = oneh^T                 (PE transpose via identity)
      tcol  = onehT.T @ trans^T      (PE: gathers trans[:, tag_{t+1}])
      cand  = (shist_t + tcol) + e_sel    (exact reference add order)
      tag_t = first-occurrence argmax_i cand, via FIND_INDEX8 fed with
              the KNOWN max: max(cand) = s_{t+1}[b, tag_{t+1}] exactly
              (monotone rounding), extracted off the critical path by a
              one-hot accumulate — no MAX8 on the serial chain.
  e_sel = e_{t+1}[b, tag_{t+1}] via a one-hot multiply-accumulate.

Exactness: reference cand[b,i,j] = (s[b,i] + trans[i,j]) + e[t,b,j], max
over i, first-occurrence argmax. Deferring the emission add past the max
is bit-exact for the max (round is monotone; e is constant over i), and
the backtrace recomputes cand in the reference's exact add order, so both
scores and tie resolution match the reference bit-for-bit. The PE gather
is exact because its lhsT is a one-hot matrix (single x*1.0 term).
"""

import numpy as np

import concourse.bacc as bacc
import concourse.mybir as mybir
import concourse.tile as tile
from concourse.bass_utils import run_bass_kernel_spmd

B, T, K = 512, 512, 64
NCORES = 8
BC = B // NCORES  # 64 batch rows per core
KH = K // 2       # 32, the j-half width

F32 = mybir.dt.float32
I32 = mybir.dt.int32
AX = mybir.AxisListType.X
OP = mybir.AluOpType

FCH = 32   # forward emission chunk, steps
BCH = 32   # backtrace emission chunk, steps


def build_nc(t_run=T):
    nc = bacc.Bacc("TRN2", target_bir_lowering=False, debug=False)

    em_split = nc.dram_tensor("em_split", [128, t_run * KH], F32,
                              kind="ExternalInput")
    em_full = nc.dram_tensor("em_full", [BC, t_run * K], F32,
                             kind="ExternalInput")
    ttsplit = nc.dram_tensor("ttsplit", [128, KH * K], F32,
                             kind="ExternalInput")
    start_split = nc.dram_tensor("start_split", [128, KH], F32,
                                 kind="ExternalInput")
    transt = nc.dram_tensor("transt", [K, K], F32, kind="ExternalInput")
    endr = nc.dram_tensor("endr", [1, K], F32, kind="ExternalInput")
    wcoefr = nc.dram_tensor("wcoefr", [1, K], F32, kind="ExternalInput")
    iotar = nc.dram_tensor("iotar", [1, K], F32, kind="ExternalInput")
    ident = nc.dram_tensor("ident", [K, K], F32, kind="ExternalInput")
    tags = nc.dram_tensor("tags", [BC, t_run], I32, kind="ExternalOutput")

    nfch = t_run // FCH
    nbch = t_run // BCH

    with tile.TileContext(nc) as tc:
        with (
            tc.tile_pool(name="persist", bufs=1) as pp,
            tc.tile_pool(name="zwork", bufs=1) as zp,
            tc.tile_pool(name="fem", bufs=2) as fep,
            tc.tile_pool(name="bem", bufs=2) as bep,
        ):
            tts = pp.tile_from(ttsplit[:, :])
            startS = pp.tile_from(start_split[:, :])
            transTS = pp.tile_from(transt[:, :])
            endS = pp.tile_from(endr[0:1, :].broadcast_to([BC, K]))
            wcoefS = pp.tile_from(wcoefr[0:1, :].broadcast_to([BC, K]))
            iotaS = pp.tile_from(iotar[0:1, :].broadcast_to([BC, K]))
            identS = pp.tile_from(ident[:, :])
            tts3 = tts[:, :].rearrange("p (j i) -> p j i", i=K)

            rawa = pp.tile([128, KH], F32)
            rawb = pp.tile([128, KH], F32)
            rawp = [rawa, rawb]
            srepa = pp.tile([128, K], F32)
            srepb = pp.tile([128, K], F32)
            srepp = [srepa, srepb]
            shist = pp.tile([BC, t_run * K], F32)   # s_0 .. s_{T-1}
            tagsf = pp.tile([BC, t_run], F32)
            tagsi = pp.tile([BC, t_run], I32)
            oneh = pp.tile([BC, K], F32)
            onehT = pp.tile([BC, K], F32)
            c1 = pp.tile([BC, K], F32)
            w1 = pp.tile([BC, K], F32)
            fin = pp.tile([BC, K], F32)
            m1 = pp.tile([BC, 1], F32)
            pw = pp.tile([BC, 1], F32)
            esel = pp.tile([BC, 1], F32)
            junk = pp.tile([BC, K], F32)
            mx8 = pp.tile([BC, 8], F32)
            junk2 = pp.tile([BC, K], F32)
            mi8 = pp.tile([BC, 8], mybir.dt.uint32)
            idxf = pp.tile([BC, 1], F32)

            # ---------------- forward ----------------
            # All-DVE forward. srep ("s replicated": every partition holds
            # the full 64-wide score vector s_t) is rebuilt each step by 4
            # quadrant tensor_adds that simultaneously apply the emission
            # add; two of them read across partition groups (cross-offset
            # APs, verified on HW). The [b,i]-layout history snapshot is a
            # single SBUF->SBUF DMA of srep's lower half, off the DVE chain.
            fetiles = {}

            def load_f(c):
                if 0 <= c < nfch and c not in fetiles:
                    ft = fep.tile([128, FCH * KH], F32, tag="fe")
                    nc.sync.dma_start(
                        ft[:, :],
                        em_split[:, c * FCH * KH:(c + 1) * FCH * KH])
                    fetiles[c] = ft

            load_f(0)
            for t in range(t_run):
                c, r = divmod(t, FCH)
                if r == 0:
                    load_f(c + 1)
                fechunk = fetiles[c]
                e_t = fechunk[:, r * KH:(r + 1) * KH]
                raw = startS if t == 0 else rawp[t % 2]
                srep = srepp[t % 2]
                # srep[p, :] = s_t (full width) = raw_t + e_t, quadrant-wise
                nc.vector.tensor_add(srep[0:BC, 0:KH], raw[0:BC, :],
                                     e_t[0:BC, :])
                nc.vector.tensor_add(srep[BC:128, KH:K], raw[BC:128, :],
                                     e_t[BC:128, :])
                nc.vector.tensor_add(srep[0:BC, KH:K], raw[BC:128, :],
                                     e_t[BC:128, :])
                nc.vector.tensor_add(srep[BC:128, 0:KH], raw[0:BC, :],
                                     e_t[0:BC, :])
                nc.sync.dma_start(shist[:, t * K:(t + 1) * K],
                                  srep[0:BC, :])
                if t < t_run - 1:
                    z = zp.tile([128, KH * K], F32, tag="z")
                    z3 = z[:, :].rearrange("p (j i) -> p j i", i=K)
                    srep_b3 = srep[:, :].unsqueeze(1).broadcast_to(
                        [128, KH, K])
                    nc.vector.tensor_add(z3, srep_b3, tts3)
                    nc.vector.tensor_reduce(rawp[(t + 1) % 2][:, :], z3,
                                            axis=AX, op=OP.max)

            nc.vector.tensor_add(fin[:, :], srepp[(t_run - 1) % 2][0:BC, :],
                                 endS[:, :])

            # ---------------- final argmax ----------------
            nc.vector.tensor_reduce(m1[:, :], fin[:, :], axis=AX, op=OP.max)
            nc.vector.scalar_tensor_tensor(
                out=w1[:, :], in0=fin[:, :], scalar=m1[:, 0:1],
                in1=wcoefS[:, :], op0=OP.is_ge, op1=OP.mult)
            nc.vector.tensor_reduce(pw[:, :], w1[:, :], axis=AX, op=OP.max)
            nc.vector.tensor_single_scalar(oneh[:, :], w1[:, :], pw[:, 0:1],
                                           op=OP.is_equal)
            nc.vector.tensor_scalar(tagsf[:, t_run - 1:t_run], pw[:, :],
                                    -1.0, 64.0, op0=OP.mult, op1=OP.add)

            # ---------------- backtrace ----------------
            betiles = {}

            def load_b(c):
                if 0 <= c < nbch and c not in betiles:
                    bt = bep.tile([BC, BCH * K], F32, tag="be")
                    nc.sync.dma_start(
                        bt[:, :],
                        em_full[:, c * BCH * K:(c + 1) * BCH * K])
                    betiles[c] = bt

            nc.vector.memset(mx8[:, :], 0.0)
            with tc.tile_pool(name="psumB", bufs=2, space="PSUM") as psb:
                load_b(nbch - 1)
                for c in range(nbch - 1, -1, -1):
                    load_b(c - 1)
                    bchunk = betiles[c]
                    t1_lo = max(c * BCH, 1)
                    for t1 in range((c + 1) * BCH - 1, t1_lo - 1, -1):
                        t = t1 - 1  # computing tag_t from tag_{t+1}=tag_{t1}
                        rr = t1 - c * BCH
                        e_t1 = bchunk[:, rr * K:(rr + 1) * K]
                        pG = psb.tile([BC, K], F32, tag="pg")
                        pT = psb.tile([BC, K], F32, tag="pt")
                        # prefill the gather PSUM with s_t (depends only on
                        # t, runs in the PE-wait bubble); the gather matmul
                        # then ACCUMULATES trans[:, tag_{t+1}] onto it
                        # (start=False), folding the c1 add into the PE.
                        nc.vector.tensor_copy(pG[:, :],
                                              shist[:, t * K:(t + 1) * K])
                        nc.tensor.transpose(pT[:, :], oneh[:, :], identS)
                        nc.vector.tensor_copy(onehT[:, :], pT[:, :])
                        nc.tensor.matmul(pG[:, :], onehT[:, :],
                                         transTS[:, :], start=False,
                                         stop=True, skip_group_check=True)
                        nc.vector.scalar_tensor_tensor(
                            out=junk[:, :], in0=oneh[:, :], scalar=1.0,
                            in1=e_t1, op0=OP.mult, op1=OP.mult,
                            accum_out=esel[:, :])
                        # known max of cand: m_sel = s_{t+1}[b, tag_{t+1}]
                        # (= round(max(c1)+esel) by monotone rounding); goes
                        # into find_index8's in_max[0], removing MAX8 from
                        # the serial chain.
                        nc.vector.scalar_tensor_tensor(
                            out=junk2[:, :], in0=oneh[:, :], scalar=1.0,
                            in1=shist[:, (t + 1) * K:(t + 2) * K],
                            op0=OP.mult, op1=OP.mult,
                            accum_out=mx8[:, 0:1])
                        nc.vector.tensor_single_scalar(
                            c1[:, :], pG[:, :], esel[:, 0:1], op=OP.add)
                        nc.vector.max_index(mi8[:, :], mx8[:, :], c1[:, :])
                        nc.vector.tensor_copy(idxf[:, :], mi8[:, 0:1])
                        nc.vector.tensor_single_scalar(
                            oneh[:, :], iotaS[:, :], idxf[:, 0:1],
                            op=OP.is_equal)
                        nc.vector.tensor_copy(tagsf[:, t:t + 1], idxf[:, :])

            nc.vector.tensor_copy(tagsi[:, :], tagsf[:, :])
            nc.sync.dma_start(tags[:, :], tagsi[:, :])

    nc.compile()
    return nc


def make_in_maps(emissions, start_transitions, end_transitions, transitions,
                 t_run=T):
    em = np.asarray(emissions, dtype=np.float32)
    start = np.asarray(start_transitions, dtype=np.float32)
    end = np.asarray(end_transitions, dtype=np.float32)
    trans = np.asarray(transitions, dtype=np.float32)

    transT = np.ascontiguousarray(trans.T)
    ttsplit = np.concatenate([
        np.tile(transT[0:KH, :].reshape(1, -1), (64, 1)),
        np.tile(transT[KH:K, :].reshape(1, -1), (64, 1)),
    ], axis=0).astype(np.float32)
    start_split = np.concatenate([
        np.tile(start[None, 0:KH], (64, 1)),
        np.tile(start[None, KH:K], (64, 1)),
    ], axis=0).astype(np.float32)

    base = {
        "ttsplit": np.ascontiguousarray(ttsplit),
        "start_split": np.ascontiguousarray(start_split),
        "transt": transT,
        "endr": np.ascontiguousarray(end[None, :]),
        "wcoefr": (K - np.arange(K, dtype=np.float32))[None, :],
        "iotar": np.arange(K, dtype=np.float32)[None, :],
        "ident": np.eye(K, dtype=np.float32),
    }
    in_maps = []
    for cix in range(NCORES):
        emc = em[cix * BC:(cix + 1) * BC, :t_run]  # [BC, t_run, K]
        m = dict(base)
        m["em_full"] = np.ascontiguousarray(emc.reshape(BC, t_run * K))
        m["em_split"] = np.ascontiguousarray(
            np.concatenate([emc[:, :, 0:KH], emc[:, :, KH:K]],
                           axis=0).reshape(128, t_run * KH))
        in_maps.append(m)
    return in_maps


def kernel(emissions, attn_mask, start_transitions, end_transitions,
           transitions):
    # attn_mask is all-ones for this problem (spec fill=ones); with an
    # all-True mask the reference's mask logic is a no-op.
    nc = build_nc(T)
    in_maps = make_in_maps(emissions, start_transitions, end_transitions,
                           transitions, T)
    res = run_bass_kernel_spmd(nc, in_maps, list(range(NCORES))).results
    out = np.concatenate([res[c]["tags"] for c in range(NCORES)], axis=0)
    return out.astype(np.int32)


if __name__ == "__main__":
    rng = np.random.default_rng(0)
    em = rng.standard_normal((B, T, K)).astype(np.float32)
    am = np.ones((B, T), np.int32)
    st = (rng.standard_normal(K) * 0.1).astype(np.float32)
    en = (rng.standard_normal(K) * 0.1).astype(np.float32)
    tr = (rng.standard_normal((K, K)) * 0.1).astype(np.float32)
    print(kernel(em, am, st, en, tr)[:2, :8])
